# revision 1
# baseline (speedup 1.0000x reference)
"""AxialAttention Trainium2 kernel (8-core data-parallel over batch).

Per image: qkv = x @ qkv_w + alpha*img; per head (16, dh=64) axial-roped
q,k; scores along W per row (no softmax); v row-summed; GroupNorm per
(b, head); output projection.

Algebraic simplifications (exact to ~1e-9 rel):
  - per-head gamma scale on k is removed by GroupNorm -> dropped.
  - height-half rope rotations cancel in q.k (same row, orthogonal) ->
    rope only on width-half features (32 of 64 per head).
  - v only needed row-summed: vsum = (A @ x) @ Wv + alpha*(A @ img) ->
    the per-token v projection is skipped entirely.

Layouts: 112-token tiles (4 rows); token-major projection -> rope ->
PE-transpose into feature-major [128=head-pair, 784] per image;
attention as S_T = Kt^T Qt per row (out at psum strip 32*b), then
outT = vsum^T @ S_T_all; GroupNorm stats via ACT accum + gpsimd all-reduce.
"""

import math
import sys

import numpy as np

for _p in ("/opt/trn_rl_repo", "/root/.axon_site/_ro/trn_rl_repo"):
    if _p not in sys.path:
        sys.path.append(_p)

import concourse.bacc as bacc
import concourse.mybir as mybir
from concourse import bass_isa, tile
from concourse.bass_utils import run_bass_kernel_spmd

F32 = mybir.dt.float32
ALU = mybir.AluOpType
ACTF = mybir.ActivationFunctionType

HEADS = 16
DH = 64
H = W = 28
HID = 1024
B_FULL = 32
N_CORES = 8
B_CORE = B_FULL // N_CORES          # 4 images per core
TOK = B_CORE * H * W                # 3136 tokens per core
TT = 112                            # tokens per tile (4 rows)
NTILES = TOK // TT                  # 28
TPI = H * W                         # 784 tokens per image
JPI = TPI // TT                     # 7 tiles per image
ALPHA = 1.0 - math.tanh(math.pi * 6.0 / 12.0)
EPS = 1e-5
NGRP = float(H * W * DH)

_CACHE = {}


def _build_program(gn_w, gn_b, stage='full'):
    nc = bacc.Bacc("TRN2", target_bir_lowering=False, debug=False,
                   num_devices=N_CORES)

    x_d = nc.dram_tensor("x", [TOK, HID], F32, kind="ExternalInput").ap()
    img_d = nc.dram_tensor("img", [TOK, HID], F32, kind="ExternalInput").ap()
    wqk_d = nc.dram_tensor("wqk", [HID, 2 * HID], F32, kind="ExternalInput").ap()
    wv_d = nc.dram_tensor("wv", [HID, HID], F32, kind="ExternalInput").ap()
    wo_d = nc.dram_tensor("wo", [HID, HID], F32, kind="ExternalInput").ap()
    at_d = nc.dram_tensor("at", [TOK, 128], F32, kind="ExternalInput").ap()
    idn_d = nc.dram_tensor("idn", [128, 128], F32, kind="ExternalInput").ap()
    ct_d = nc.dram_tensor("ctab", [TPI, 512], F32, kind="ExternalInput").ap()
    st_d = nc.dram_tensor("stab", [TPI, 512], F32, kind="ExternalInput").ap()
    y_d = nc.dram_tensor("y", [TOK, HID], F32, kind="ExternalOutput").ap()

    from contextlib import ExitStack
    with ExitStack() as ctx:
        tc = ctx.enter_context(tile.TileContext(nc))
        constp = ctx.enter_context(tc.tile_pool(name="const", bufs=1))
        wqkp = ctx.enter_context(tc.tile_pool(name="wqk", bufs=1))
        wop = ctx.enter_context(tc.tile_pool(name="wo", bufs=1))
        xinp = ctx.enter_context(tc.tile_pool(name="xin", bufs=2))
        imgp = ctx.enter_context(tc.tile_pool(name="imgin", bufs=2))
        xtp = ctx.enter_context(tc.tile_pool(name="xt", bufs=2))
        qkp = ctx.enter_context(tc.tile_pool(name="qk", bufs=2))
        tabp = ctx.enter_context(tc.tile_pool(name="tab", bufs=1))
        rtp = ctx.enter_context(tc.tile_pool(name="rt", bufs=1))
        bigp = ctx.enter_context(tc.tile_pool(name="big", bufs=17))
        stsbp = ctx.enter_context(tc.tile_pool(name="stsb", bufs=2))
        smallp = ctx.enter_context(tc.tile_pool(name="small", bufs=1))
        accp = ctx.enter_context(tc.tile_pool(name="acc", bufs=1))
        youtp = ctx.enter_context(tc.tile_pool(name="yout", bufs=2))
        pqp = ctx.enter_context(tc.tile_pool(name="pq", bufs=2, space="PSUM"))
        ptrp = ctx.enter_context(tc.tile_pool(name="ptr", bufs=1, space="PSUM"))
        pstp = ctx.enter_context(tc.tile_pool(name="pst", bufs=1, space="PSUM"))
        potp = ctx.enter_context(tc.tile_pool(name="pot", bufs=1, space="PSUM"))
        if True:
            idn = constp.tile([128, 128], F32, tag="idn")
            nc.sync.dma_start(idn[:], idn_d[:])
            gw = constp.tile([128, 32], F32, tag="gw")
            epsb = constp.tile([128, 1], F32, tag="epsb")
            nc.gpsimd.memset(epsb[:], EPS)
            for n in range(HEADS):
                nc.gpsimd.memset(gw[:, n:n + 1], float(gn_w[n]))
                nc.gpsimd.memset(gw[:, 16 + n:17 + n], float(gn_b[n]))

            wqk_sb = []
            for k in range(8):
                t = wqkp.tile([128, 2 * HID], F32, tag=f"wqk{k}", name=f"wqk_sb{k}")
                nc.sync.dma_start(t[:], wqk_d[128 * k:128 * (k + 1), :])
                wqk_sb.append(t)
            wo_sb = []
            for k in range(8):
                t = wop.tile([128, HID], F32, tag=f"wo{k}", name=f"wo_sb{k}")
                nc.sync.dma_start(t[:], wo_d[128 * k:128 * (k + 1), :])
                wo_sb.append(t)

            # ---------------- phase 0: row-sums + vsum ----------------
            vsum = smallp.tile([128, HID], F32, tag="vsum")
            with tc.tile_pool(name="ph0", bufs=1) as ph0p, \
                 tc.tile_pool(name="ph0at", bufs=2) as atp:
                xs_ps = pstp.tile([128, HID], F32, tag="st_ps", name="xs_ps")
                is_ps = potp.tile([128, HID], F32, tag="ot_ps", name="is_ps")
                for i in range(NTILES):
                    rs = slice(TT * i, TT * (i + 1))
                    xt0 = xinp.tile([TT, HID], F32, tag="x0")
                    nc.sync.dma_start(xt0[:], x_d[rs, :])
                    it0 = imgp.tile([TT, HID], F32, tag="i0")
                    nc.sync.dma_start(it0[:], img_d[rs, :])
                    att = atp.tile([TT, 128], F32, tag="at")
                    nc.sync.dma_start(att[:], at_d[rs, :])
                    for n in range(2):
                        cs = slice(512 * n, 512 * (n + 1))
                        nc.tensor.matmul(xs_ps[:, cs], att[:], xt0[:, cs],
                                         start=(i == 0), stop=(i == NTILES - 1))
                        nc.tensor.matmul(is_ps[:, cs], att[:], it0[:, cs],
                                         start=(i == 0), stop=(i == NTILES - 1))
                xs_sb = ph0p.tile([128, HID], F32, tag="xs_sb")
                nc.vector.tensor_copy(xs_sb[:], xs_ps[:])
                is_sb = ph0p.tile([128, HID], F32, tag="is_sb")
                nc.vector.tensor_copy(is_sb[:], is_ps[:])
                xsT = xtp.tile([128, HID], F32, tag="xts", name="xsT")
                for k in range(8):
                    tp = ptrp.tile([128, 128], F32, tag="tr")
                    nc.tensor.transpose(tp[:], xs_sb[:, 128 * k:128 * (k + 1)],
                                        idn[:, :])
                    nc.vector.tensor_copy(xsT[:, 128 * k:128 * (k + 1)], tp[:])
                vs_a = pqp.tile([TT, 512], F32, tag="pq", name="vs_a",
                                padded_shape=None) if False else pqp.tile(
                    [128, 512], F32, tag="pq", name="vs_a")
                vs_b = pqp.tile([128, 512], F32, tag="pq", name="vs_b")
                for k in range(8):
                    wvt = imgp.tile([128, HID], F32, tag="i0", name=f"wvt{k}")
                    nc.sync.dma_start(wvt[:], wv_d[128 * k:128 * (k + 1), :])
                    nc.tensor.matmul(vs_a[:],
                                     xsT[:, 128 * k:128 * (k + 1)],
                                     wvt[:, 0:512], start=(k == 0), stop=(k == 7))
                    nc.tensor.matmul(vs_b[:],
                                     xsT[:, 128 * k:128 * (k + 1)],
                                     wvt[:, 512:1024], start=(k == 0), stop=(k == 7))
                nc.vector.scalar_tensor_tensor(vsum[:, 0:512], is_sb[:, 0:512],
                                               ALPHA, vs_a[:], ALU.mult, ALU.add)
                nc.vector.scalar_tensor_tensor(vsum[:, 512:1024], is_sb[:, 512:1024],
                                               ALPHA, vs_b[:], ALU.mult, ALU.add)

            if stage == 'vsum':
                nc.sync.dma_start(y_d[0:128, :], vsum[:])
                b_range = []
            else:
                b_range = list(range(B_CORE))
            # ---------------- per-image pipeline ----------------
            for b in b_range:
                bb = 32 * b
                qf = [bigp.tile([128, TPI], F32, tag="big", name=f"qf{b}_{_i}") for _i in range(8)]
                kf = [bigp.tile([128, TPI], F32, tag="big", name=f"kf{b}_{_i}") for _i in range(8)]
                for j in range(JPI):
                    i = JPI * b + j
                    rs = slice(TT * i, TT * (i + 1))
                    xt1 = xinp.tile([TT, HID], F32, tag="x0")
                    nc.sync.dma_start(xt1[:], x_d[rs, :])
                    it1 = imgp.tile([TT, HID], F32, tag="i0")
                    nc.sync.dma_start(it1[:], img_d[rs, :])
                    xts = xtp.tile([128, 8 * TT], F32, tag="xts")
                    for k in range(8):
                        tp = ptrp.tile([128, 128], F32, tag="tr")
                        nc.tensor.transpose(tp[:, 0:TT],
                                            xt1[:, 128 * k:128 * (k + 1)],
                                            idn[0:TT, 0:TT])
                        nc.vector.tensor_copy(xts[:, TT * k:TT * (k + 1)],
                                              tp[:, 0:TT])
                    ct = tabp.tile([TT, 512], F32, tag="ct")
                    nc.sync.dma_start(ct[:], ct_d[TT * j:TT * (j + 1), :])
                    st = tabp.tile([TT, 512], F32, tag="st")
                    nc.sync.dma_start(st[:], st_d[TT * j:TT * (j + 1), :])
                    for n in range(4):   # 512-col chunks of [q | k]
                        pq = pqp.tile([TT, 512], F32, tag="pq")
                        for k in range(8):
                            nc.tensor.matmul(pq[:],
                                             xts[:, TT * k:TT * (k + 1)],
                                             wqk_sb[k][:, 512 * n:512 * (n + 1)],
                                             start=(k == 0), stop=(k == 7))
                        qkc = qkp.tile([TT, 512], F32, tag="qkc")
                        nc.vector.scalar_tensor_tensor(
                            qkc[:], it1[:, 512 * (n % 2):512 * (n % 2 + 1)],
                            ALPHA, pq[:], ALU.mult, ALU.add)
                        # rope on width-halves (8 heads per chunk)
                        hh = 8 * (n % 2)
                        qv = qkc[:].rearrange("p (h d) -> p h d", d=64)[:, :, 32:64]
                        cv = ct[:].rearrange("p (h d) -> p h d", d=32)[:, hh:hh + 8, :]
                        sv = st[:].rearrange("p (h d) -> p h d", d=32)[:, hh:hh + 8, :]
                        t1 = rtp.tile([TT, 256], F32, tag="t1")
                        t1v = t1[:].rearrange("p (h d) -> p h d", d=32)
                        t2 = rtp.tile([TT, 256], F32, tag="t2")
                        t2v = t2[:].rearrange("p (h d) -> p h d", d=32)
                        nc.vector.tensor_tensor(t1v[:], qv[:], cv[:], op=ALU.mult)
                        nc.vector.tensor_tensor(t2v[:, :, 0:16], qv[:, :, 16:32],
                                                sv[:, :, 0:16], op=ALU.mult)
                        nc.vector.tensor_tensor(t2v[:, :, 16:32], qv[:, :, 0:16],
                                                sv[:, :, 16:32], op=ALU.mult)
                        nc.vector.tensor_tensor(qv[:], t1v[:], t2v[:], op=ALU.add)
                        for c in range(4):
                            cg = 4 * n + c
                            tp = ptrp.tile([128, 128], F32, tag="tr")
                            nc.tensor.transpose(tp[:, 0:TT],
                                                qkc[:, 128 * c:128 * (c + 1)],
                                                idn[0:TT, 0:TT])
                            dst = qf[cg] if cg < 8 else kf[cg - 8]
                            nc.scalar.copy(dst[:, TT * j:TT * (j + 1)],
                                           tp[:, 0:TT])

                if stage == 'proj':
                    for p in range(8):
                        nc.sync.dma_start(
                            y_d[TPI * 0 + 0:0 + 784, 0:128].rearrange("t c -> c t") if False else y_d[0:128, 0:784 // 2],
                            qf[p][:, 0:392])
                    continue
                # ---- attention + stats for image b ----
                statb = accp.tile([128, 32], F32, tag="statb")
                nc.gpsimd.memset(statb[:], 0.0)
                of = []
                for p in range(8):
                    ofp = bigp.tile([128, TPI], F32, tag="big", name=f"of{b}_{p}")
                    of.append(ofp)
                    for hn in range(2):
                        n = 2 * p + hn
                        hb = 64 * hn
                        stp = pstp.tile([128, 896], F32, tag="st_ps")
                        for r in range(28):
                            nc.tensor.matmul(stp[bb:bb + 28, 32 * r:32 * r + 28],
                                             kf[p][hb:hb + 64, 28 * r:28 * (r + 1)],
                                             qf[p][hb:hb + 64, 28 * r:28 * (r + 1)],
                                             tile_position=(hb, bb),
                                             start=True, stop=True)
                        st_sb = stsbp.tile([128, TPI], F32, tag="st_sb")
                        stv = stp[bb:bb + 28, :].rearrange(
                            "p (r c) -> p r c", c=32)[:, :, 0:28]
                        nc.vector.tensor_copy(
                            st_sb[bb:bb + 28, :].rearrange(
                                "p (r c) -> p r c", c=28), stv)
                        otp = potp.tile([128, TPI], F32, tag="ot_ps")
                        nc.tensor.matmul(otp[hb:hb + 64, 0:512],
                                         vsum[bb:bb + 28, 64 * n:64 * (n + 1)],
                                         st_sb[bb:bb + 28, 0:512],
                                         tile_position=(bb, hb),
                                         start=True, stop=True)
                        nc.tensor.matmul(otp[hb:hb + 64, 512:TPI],
                                         vsum[bb:bb + 28, 64 * n:64 * (n + 1)],
                                         st_sb[bb:bb + 28, 512:TPI],
                                         tile_position=(bb, hb),
                                         start=True, stop=True)
                        nc.scalar.activation(ofp[hb:hb + 64, :],
                                             otp[hb:hb + 64, :], ACTF.Copy,
                                             accum_out=statb[hb:hb + 64,
                                                             n:n + 1])
                        sqt = stsbp.tile([128, TPI], F32, tag="st_sb", name=f"sq{b}_{p}_{hn}")
                        nc.scalar.activation(sqt[hb:hb + 64, :],
                                             ofp[hb:hb + 64, :], ACTF.Square,
                                             accum_out=statb[hb:hb + 64,
                                                             16 + n:17 + n])

                if stage not in ('nostat',):
                    allred = accp.tile([128, 32], F32, tag="allred")
                    nc.gpsimd.partition_all_reduce(
                        allred[:], statb[:], channels=128,
                        reduce_op=bass_isa.ReduceOp.add)
                    m2 = accp.tile([128, 32], F32, tag="m2")
                    nc.scalar.mul(m2[:], allred[:], 1.0 / NGRP)
                    msq = accp.tile([128, 16], F32, tag="msq")
                    nc.scalar.activation(msq[:], m2[:, 0:16], ACTF.Square)
                    var = accp.tile([128, 16], F32, tag="var")
                    nc.vector.tensor_tensor(var[:], m2[:, 16:32], msq[:],
                                            op=ALU.subtract)
                    sd = accp.tile([128, 16], F32, tag="sd")
                    nc.scalar.activation(sd[:], var[:], ACTF.Sqrt,
                                         bias=epsb[:, 0:1])
                    inv = accp.tile([128, 16], F32, tag="inv")
                    nc.vector.reciprocal(inv[:], sd[:])
                    acsb = accp.tile([128, 32], F32, tag="acsb")
                    nc.vector.tensor_tensor(acsb[:, 0:16], inv[:],
                                            gw[:, 0:16], op=ALU.mult)
                    ctmp = accp.tile([128, 16], F32, tag="ctmp")
                    nc.vector.scalar_tensor_tensor(ctmp[:], m2[:, 0:16],
                                                   -1.0, acsb[:, 0:16],
                                                   ALU.mult, ALU.mult)
                    nc.vector.tensor_tensor(acsb[:, 16:32], ctmp[:],
                                            gw[:, 16:32], op=ALU.add)
                    of2 = []
                    for p in range(8):
                        of2p = bigp.tile([128, TPI], F32, tag="big",
                                         name=f"of2_{b}_{p}")
                        of2.append(of2p)
                        for hn in range(2):
                            n = 2 * p + hn
                            hb = 64 * hn
                            nc.scalar.activation(
                                of2p[hb:hb + 64, :], of[p][hb:hb + 64, :],
                                ACTF.Identity,
                                scale=acsb[hb:hb + 64, n:n + 1],
                                bias=acsb[hb:hb + 64, 16 + n:17 + n])
                    of = of2
                # ---- output projection for image b ----
                if stage == 'attn':
                    for p in range(8):
                        nc.sync.dma_start(y_d[128 * 0:128 * 0 + 128, 0:784], of[p][:])
                    continue
                for j in range(JPI):
                    ts = slice(TT * j, TT * (j + 1))
                    for nn in range(2):
                        yp = pqp.tile([TT, 512], F32, tag="pq")
                        for k in range(8):
                            nc.tensor.matmul(yp[:], of[k][:, ts],
                                             wo_sb[k][:, 512 * nn:512 * (nn + 1)],
                                             start=(k == 0), stop=(k == 7))
                        y_sb = youtp.tile([TT, 512], F32, tag="y_sb")
                        nc.vector.tensor_copy(y_sb[:], yp[:])
                        nc.sync.dma_start(
                            y_d[TPI * b + TT * j:TPI * b + TT * (j + 1),
                                512 * nn:512 * (nn + 1)], y_sb[:])
    nc.compile()
    return nc


def _host_tables():
    inv_freq = 1.0 / (10000.0 ** (np.arange(0, 16, dtype=np.float64) * 2 / 32))
    wpos = np.arange(W, dtype=np.float64)
    ang = wpos[:, None] * inv_freq[None, :]          # [28, 16]
    cosw = np.cos(ang).astype(np.float32)
    sinw = np.sin(ang).astype(np.float32)
    # per-token (within image) tables, replicated per head:
    # C block = [cos, cos]; S block = [-sin, +sin]
    cblk = np.concatenate([cosw, cosw], axis=1)       # [28, 32]
    sblk = np.concatenate([-sinw, sinw], axis=1)      # [28, 32]
    crow = np.tile(cblk, (1, HEADS))                  # [28, 512]
    srow = np.tile(sblk, (1, HEADS))
    ctab = np.tile(crow, (H, 1)).reshape(TPI, 512)    # rows t=r*28+w -> w pattern
    stab = np.tile(srow, (H, 1)).reshape(TPI, 512)
    # careful: np.tile(crow, (H,1)) stacks w-rows H times: row t = t%28 ✓
    at = np.zeros((TOK, 128), dtype=np.float32)
    t = np.arange(TOK)
    at[t, 32 * (t // TPI) + (t % W)] = 1.0
    idn = np.eye(128, dtype=np.float32)
    return ctab, stab, at, idn


def kernel(x, input_img, qkv_w, o_w, gn_w, gn_b):
    x = np.ascontiguousarray(np.asarray(x, dtype=np.float32))
    input_img = np.ascontiguousarray(np.asarray(input_img, dtype=np.float32))
    qkv_w = np.asarray(qkv_w, dtype=np.float32)
    o_w = np.ascontiguousarray(np.asarray(o_w, dtype=np.float32))
    gn_w = np.asarray(gn_w, dtype=np.float32)
    gn_b = np.asarray(gn_b, dtype=np.float32)

    key = (tuple(gn_w.tolist()), tuple(gn_b.tolist()))
    if key not in _CACHE:
        _CACHE[key] = _build_program(gn_w, gn_b)
    nc = _CACHE[key]

    ctab, stab, at, idn = _host_tables()
    wqk = np.ascontiguousarray(
        np.concatenate([qkv_w[:, 0:HID], qkv_w[:, 2 * HID:3 * HID]], axis=1))
    wv = np.ascontiguousarray(qkv_w[:, HID:2 * HID])

    in_maps = []
    for c in range(N_CORES):
        in_maps.append({
            "x": x[B_CORE * c:B_CORE * (c + 1)].reshape(TOK, HID),
            "img": input_img[B_CORE * c:B_CORE * (c + 1)].reshape(TOK, HID),
            "wqk": wqk, "wv": wv, "wo": o_w,
            "at": at, "idn": idn, "ctab": ctab, "stab": stab,
        })
    res = run_bass_kernel_spmd(nc, in_maps, list(range(N_CORES)))
    out = np.concatenate(
        [res.results[c]["y"].reshape(B_CORE, H, W, HID)
         for c in range(N_CORES)], axis=0)
    return out



# revision 3
# speedup vs baseline: 2.3820x; 2.3820x over previous
"""AxialAttention Trainium2 kernel (8-core data-parallel over batch).

Per image: qkv = x @ qkv_w + alpha*img; per head (16, dh=64) axial-roped
q,k; scores along W per row (no softmax); v row-summed; GroupNorm per
(b, head); output projection.

Algebraic simplifications (exact to ~1e-9 rel):
  - per-head gamma scale on k is removed by GroupNorm -> dropped.
  - height-half rope rotations cancel in q.k (same row, orthogonal) ->
    rope only on width-half features (32 of 64 per head).
  - v only needed row-summed: vsum = (A @ x) @ Wv + alpha*(A @ img) ->
    the per-token v projection is skipped entirely.

All matmul operands in bf16 (1 cycle/row on PE vs 4 for fp32), fp32 PSUM
accumulation. Inputs/weights/output converted to bf16 on host (halves DMA).
"""

import math
import sys

import numpy as np
import ml_dtypes

for _p in ("/opt/trn_rl_repo", "/root/.axon_site/_ro/trn_rl_repo"):
    if _p not in sys.path:
        sys.path.append(_p)

import concourse.bacc as bacc
import concourse.mybir as mybir
from concourse import bass_isa, tile
from concourse.bass_utils import run_bass_kernel_spmd

F32 = mybir.dt.float32
BF16 = mybir.dt.bfloat16
ALU = mybir.AluOpType
ACTF = mybir.ActivationFunctionType
NPBF = ml_dtypes.bfloat16

HEADS = 16
DH = 64
H = W = 28
HID = 1024
B_FULL = 32
N_CORES = 8
B_CORE = B_FULL // N_CORES          # 4 images per core
TOK = B_CORE * H * W                # 3136 tokens per core
TT = 112                            # tokens per tile (4 rows)
NTILES = TOK // TT                  # 28
TPI = H * W                         # 784 tokens per image
JPI = TPI // TT                     # 7 tiles per image
ALPHA = 1.0 - math.tanh(math.pi * 6.0 / 12.0)
EPS = 1e-5
NGRP = float(H * W * DH)

_CACHE = {}


def _build_program(gn_w, gn_b, stage='full'):
    nc = bacc.Bacc("TRN2", target_bir_lowering=False, debug=False,
                   num_devices=N_CORES)

    x_d = nc.dram_tensor("x", [TOK, HID], BF16, kind="ExternalInput").ap()
    img_d = nc.dram_tensor("img", [TOK, HID], BF16, kind="ExternalInput").ap()
    wqk_d = nc.dram_tensor("wqk", [HID, 2 * HID], BF16, kind="ExternalInput").ap()
    wv_d = nc.dram_tensor("wv", [HID, HID], BF16, kind="ExternalInput").ap()
    wo_d = nc.dram_tensor("wo", [HID, HID], BF16, kind="ExternalInput").ap()
    at_d = nc.dram_tensor("at", [TOK, 128], BF16, kind="ExternalInput").ap()
    idn_d = nc.dram_tensor("idn", [128, 128], BF16, kind="ExternalInput").ap()
    ct_d = nc.dram_tensor("ctab", [TPI, 512], BF16, kind="ExternalInput").ap()
    st_d = nc.dram_tensor("stab", [TPI, 512], BF16, kind="ExternalInput").ap()
    y_d = nc.dram_tensor("y", [TOK, HID], BF16, kind="ExternalOutput").ap()

    from contextlib import ExitStack
    with ExitStack() as ctx:
        tc = ctx.enter_context(tile.TileContext(nc))
        constp = ctx.enter_context(tc.tile_pool(name="const", bufs=1))
        wqkp = ctx.enter_context(tc.tile_pool(name="wqk", bufs=1))
        wop = ctx.enter_context(tc.tile_pool(name="wo", bufs=1))
        xinp = ctx.enter_context(tc.tile_pool(name="xin", bufs=2))
        imgp = ctx.enter_context(tc.tile_pool(name="imgin", bufs=2))
        xtp = ctx.enter_context(tc.tile_pool(name="xt", bufs=2))
        qkp = ctx.enter_context(tc.tile_pool(name="qk", bufs=2))
        tabp = ctx.enter_context(tc.tile_pool(name="tab", bufs=1))
        rtp = ctx.enter_context(tc.tile_pool(name="rt", bufs=1))
        bigp = ctx.enter_context(tc.tile_pool(name="big", bufs=17))
        stsbp = ctx.enter_context(tc.tile_pool(name="stsb", bufs=2))
        smallp = ctx.enter_context(tc.tile_pool(name="small", bufs=1))
        accp = ctx.enter_context(tc.tile_pool(name="acc", bufs=1))
        youtp = ctx.enter_context(tc.tile_pool(name="yout", bufs=2))
        pqp = ctx.enter_context(tc.tile_pool(name="pq", bufs=2, space="PSUM"))
        ptrp = ctx.enter_context(tc.tile_pool(name="ptr", bufs=1, space="PSUM"))
        pstp = ctx.enter_context(tc.tile_pool(name="pst", bufs=1, space="PSUM"))
        potp = ctx.enter_context(tc.tile_pool(name="pot", bufs=1, space="PSUM"))
        if True:
            idn = constp.tile([128, 128], BF16, tag="idn")
            nc.sync.dma_start(idn[:], idn_d[:])
            gw = constp.tile([128, 32], F32, tag="gw")
            epsb = constp.tile([128, 1], F32, tag="epsb")
            nc.gpsimd.memset(epsb[:], EPS)
            for n in range(HEADS):
                nc.gpsimd.memset(gw[:, n:n + 1], float(gn_w[n]))
                nc.gpsimd.memset(gw[:, 16 + n:17 + n], float(gn_b[n]))

            wqk_sb = []
            for k in range(8):
                t = wqkp.tile([128, 2 * HID], BF16, tag=f"wqk{k}", name=f"wqk_sb{k}")
                nc.sync.dma_start(t[:], wqk_d[128 * k:128 * (k + 1), :])
                wqk_sb.append(t)
            wo_sb = []
            for k in range(8):
                t = wop.tile([128, HID], BF16, tag=f"wo{k}", name=f"wo_sb{k}")
                nc.sync.dma_start(t[:], wo_d[128 * k:128 * (k + 1), :])
                wo_sb.append(t)
            # rope tables resident (reused across the 4 images)
            cts, sts = [], []
            for j in range(JPI):
                ct = tabp.tile([TT, 512], BF16, tag=f"ct{j}", name=f"ct{j}")
                nc.sync.dma_start(ct[:], ct_d[TT * j:TT * (j + 1), :])
                st = tabp.tile([TT, 512], BF16, tag=f"st{j}", name=f"st{j}")
                nc.sync.dma_start(st[:], st_d[TT * j:TT * (j + 1), :])
                cts.append(ct)
                sts.append(st)

            # ---------------- phase 0: row-sums + vsum ----------------
            vsum = smallp.tile([128, HID], BF16, tag="vsum")
            with tc.tile_pool(name="ph0", bufs=1) as ph0p, \
                 tc.tile_pool(name="ph0at", bufs=2) as atp:
                xs_ps = pstp.tile([128, HID], F32, tag="st_ps", name="xs_ps")
                is_ps = potp.tile([128, HID], F32, tag="ot_ps", name="is_ps")
                for i in range(NTILES):
                    rs = slice(TT * i, TT * (i + 1))
                    xt0 = xinp.tile([TT, HID], BF16, tag="x0")
                    nc.sync.dma_start(xt0[:], x_d[rs, :])
                    it0 = imgp.tile([TT, HID], BF16, tag="i0")
                    nc.sync.dma_start(it0[:], img_d[rs, :])
                    att = atp.tile([TT, 128], BF16, tag="at")
                    nc.sync.dma_start(att[:], at_d[rs, :])
                    for n in range(2):
                        cs = slice(512 * n, 512 * (n + 1))
                        nc.tensor.matmul(xs_ps[:, cs], att[:], xt0[:, cs],
                                         start=(i == 0), stop=(i == NTILES - 1))
                        nc.tensor.matmul(is_ps[:, cs], att[:], it0[:, cs],
                                         start=(i == 0), stop=(i == NTILES - 1))
                xs_sb = ph0p.tile([128, HID], BF16, tag="xs_sb")
                nc.vector.tensor_copy(xs_sb[:], xs_ps[:])
                is_sb = ph0p.tile([128, HID], BF16, tag="is_sb")
                nc.vector.tensor_copy(is_sb[:], is_ps[:])
                xsT = xtp.tile([128, HID], BF16, tag="xts", name="xsT")
                for k in range(8):
                    tp = ptrp.tile([128, 128], BF16, tag="tr")
                    nc.tensor.transpose(tp[:], xs_sb[:, 128 * k:128 * (k + 1)],
                                        idn[:, :])
                    nc.vector.tensor_copy(xsT[:, 128 * k:128 * (k + 1)], tp[:])
                vs_a = pqp.tile([128, 512], F32, tag="pq", name="vs_a")
                vs_b = pqp.tile([128, 512], F32, tag="pq", name="vs_b")
                for k in range(8):
                    wvt = imgp.tile([128, HID], BF16, tag="i0", name=f"wvt{k}")
                    nc.sync.dma_start(wvt[:], wv_d[128 * k:128 * (k + 1), :])
                    nc.tensor.matmul(vs_a[:],
                                     xsT[:, 128 * k:128 * (k + 1)],
                                     wvt[:, 0:512], start=(k == 0), stop=(k == 7))
                    nc.tensor.matmul(vs_b[:],
                                     xsT[:, 128 * k:128 * (k + 1)],
                                     wvt[:, 512:1024], start=(k == 0), stop=(k == 7))
                nc.vector.scalar_tensor_tensor(vsum[:, 0:512], is_sb[:, 0:512],
                                               ALPHA, vs_a[:], ALU.mult, ALU.add)
                nc.vector.scalar_tensor_tensor(vsum[:, 512:1024], is_sb[:, 512:1024],
                                               ALPHA, vs_b[:], ALU.mult, ALU.add)

            if stage == 'vsum':
                nc.sync.dma_start(y_d[0:128, :], vsum[:])
                b_range = []
            else:
                b_range = list(range(B_CORE))
            # ---------------- per-image pipeline ----------------
            for b in b_range:
                bb = 32 * b
                qf = [bigp.tile([128, TPI], BF16, tag="big", name=f"qf{b}_{_i}") for _i in range(8)]
                kf = [bigp.tile([128, TPI], BF16, tag="big", name=f"kf{b}_{_i}") for _i in range(8)]
                for j in range(JPI):
                    i = JPI * b + j
                    rs = slice(TT * i, TT * (i + 1))
                    xt1 = xinp.tile([TT, HID], BF16, tag="x0")
                    nc.sync.dma_start(xt1[:], x_d[rs, :])
                    it1 = imgp.tile([TT, HID], BF16, tag="i0")
                    nc.sync.dma_start(it1[:], img_d[rs, :])
                    xts = xtp.tile([128, 8 * TT], BF16, tag="xts")
                    for k in range(8):
                        tp = ptrp.tile([128, 128], BF16, tag="tr")
                        nc.tensor.transpose(tp[:, 0:TT],
                                            xt1[:, 128 * k:128 * (k + 1)],
                                            idn[0:TT, 0:TT])
                        nc.vector.tensor_copy(xts[:, TT * k:TT * (k + 1)],
                                              tp[:, 0:TT])
                    ct = cts[j]
                    st = sts[j]
                    for n in range(4):   # 512-col chunks of [q | k]
                        pq = pqp.tile([TT, 512], F32, tag="pq")
                        for k in range(8):
                            nc.tensor.matmul(pq[:],
                                             xts[:, TT * k:TT * (k + 1)],
                                             wqk_sb[k][:, 512 * n:512 * (n + 1)],
                                             start=(k == 0), stop=(k == 7))
                        qkc = qkp.tile([TT, 512], BF16, tag="qkc")
                        nc.vector.scalar_tensor_tensor(
                            qkc[:], it1[:, 512 * (n % 2):512 * (n % 2 + 1)],
                            ALPHA, pq[:], ALU.mult, ALU.add)
                        # rope on width-halves (8 heads per chunk)
                        hh = 8 * (n % 2)
                        qv = qkc[:].rearrange("p (h d) -> p h d", d=64)[:, :, 32:64]
                        cv = ct[:].rearrange("p (h d) -> p h d", d=32)[:, hh:hh + 8, :]
                        sv = st[:].rearrange("p (h d) -> p h d", d=32)[:, hh:hh + 8, :]
                        t1 = rtp.tile([TT, 256], BF16, tag="t1")
                        t1v = t1[:].rearrange("p (h d) -> p h d", d=32)
                        t2 = rtp.tile([TT, 256], BF16, tag="t2")
                        t2v = t2[:].rearrange("p (h d) -> p h d", d=32)
                        nc.vector.tensor_tensor(t1v[:], qv[:], cv[:], op=ALU.mult)
                        nc.vector.tensor_tensor(t2v[:, :, 0:16], qv[:, :, 16:32],
                                                sv[:, :, 0:16], op=ALU.mult)
                        nc.vector.tensor_tensor(t2v[:, :, 16:32], qv[:, :, 0:16],
                                                sv[:, :, 16:32], op=ALU.mult)
                        nc.vector.tensor_tensor(qv[:], t1v[:], t2v[:], op=ALU.add)
                        for c in range(4):
                            cg = 4 * n + c
                            tp = ptrp.tile([128, 128], BF16, tag="tr")
                            nc.tensor.transpose(tp[:, 0:TT],
                                                qkc[:, 128 * c:128 * (c + 1)],
                                                idn[0:TT, 0:TT])
                            dst = qf[cg] if cg < 8 else kf[cg - 8]
                            nc.scalar.copy(dst[:, TT * j:TT * (j + 1)],
                                           tp[:, 0:TT])

                if stage == 'proj':
                    for p in range(8):
                        nc.sync.dma_start(y_d[0:128, 0:784 // 2], qf[p][:, 0:392])
                    continue
                # ---- attention + stats for image b ----
                statb = accp.tile([128, 32], F32, tag="statb")
                nc.gpsimd.memset(statb[:], 0.0)
                of = []
                for p in range(8):
                    ofp = bigp.tile([128, TPI], BF16, tag="big", name=f"of{b}_{p}")
                    of.append(ofp)
                    for hn in range(2):
                        n = 2 * p + hn
                        hb = 64 * hn
                        stp = pstp.tile([128, 896], F32, tag="st_ps")
                        for r in range(28):
                            nc.tensor.matmul(stp[bb:bb + 28, 32 * r:32 * r + 28],
                                             kf[p][hb:hb + 64, 28 * r:28 * (r + 1)],
                                             qf[p][hb:hb + 64, 28 * r:28 * (r + 1)],
                                             tile_position=(hb, bb),
                                             start=True, stop=True)
                        st_sb = stsbp.tile([128, TPI], BF16, tag="st_sb")
                        stv = stp[bb:bb + 28, :].rearrange(
                            "p (r c) -> p r c", c=32)[:, :, 0:28]
                        nc.vector.tensor_copy(
                            st_sb[bb:bb + 28, :].rearrange(
                                "p (r c) -> p r c", c=28), stv)
                        otp = potp.tile([128, TPI], F32, tag="ot_ps")
                        nc.tensor.matmul(otp[hb:hb + 64, 0:512],
                                         vsum[bb:bb + 28, 64 * n:64 * (n + 1)],
                                         st_sb[bb:bb + 28, 0:512],
                                         tile_position=(bb, hb),
                                         start=True, stop=True)
                        nc.tensor.matmul(otp[hb:hb + 64, 512:TPI],
                                         vsum[bb:bb + 28, 64 * n:64 * (n + 1)],
                                         st_sb[bb:bb + 28, 512:TPI],
                                         tile_position=(bb, hb),
                                         start=True, stop=True)
                        nc.scalar.activation(ofp[hb:hb + 64, :],
                                             otp[hb:hb + 64, :], ACTF.Copy,
                                             accum_out=statb[hb:hb + 64,
                                                             n:n + 1])
                        sqt = stsbp.tile([128, TPI], BF16, tag="st_sb", name=f"sq{b}_{p}_{hn}")
                        nc.scalar.activation(sqt[hb:hb + 64, :],
                                             ofp[hb:hb + 64, :], ACTF.Square,
                                             accum_out=statb[hb:hb + 64,
                                                             16 + n:17 + n])

                if stage not in ('nostat',):
                    allred = accp.tile([128, 32], F32, tag="allred")
                    nc.gpsimd.partition_all_reduce(
                        allred[:], statb[:], channels=128,
                        reduce_op=bass_isa.ReduceOp.add)
                    m2 = accp.tile([128, 32], F32, tag="m2")
                    nc.scalar.mul(m2[:], allred[:], 1.0 / NGRP)
                    msq = accp.tile([128, 16], F32, tag="msq")
                    nc.scalar.activation(msq[:], m2[:, 0:16], ACTF.Square)
                    var = accp.tile([128, 16], F32, tag="var")
                    nc.vector.tensor_tensor(var[:], m2[:, 16:32], msq[:],
                                            op=ALU.subtract)
                    sd = accp.tile([128, 16], F32, tag="sd")
                    nc.scalar.activation(sd[:], var[:], ACTF.Sqrt,
                                         bias=epsb[:, 0:1])
                    inv = accp.tile([128, 16], F32, tag="inv")
                    nc.vector.reciprocal(inv[:], sd[:])
                    acsb = accp.tile([128, 32], F32, tag="acsb")
                    nc.vector.tensor_tensor(acsb[:, 0:16], inv[:],
                                            gw[:, 0:16], op=ALU.mult)
                    ctmp = accp.tile([128, 16], F32, tag="ctmp")
                    nc.vector.scalar_tensor_tensor(ctmp[:], m2[:, 0:16],
                                                   -1.0, acsb[:, 0:16],
                                                   ALU.mult, ALU.mult)
                    nc.vector.tensor_tensor(acsb[:, 16:32], ctmp[:],
                                            gw[:, 16:32], op=ALU.add)
                    of2 = []
                    for p in range(8):
                        of2p = bigp.tile([128, TPI], BF16, tag="big",
                                         name=f"of2_{b}_{p}")
                        of2.append(of2p)
                        for hn in range(2):
                            n = 2 * p + hn
                            hb = 64 * hn
                            nc.scalar.activation(
                                of2p[hb:hb + 64, :], of[p][hb:hb + 64, :],
                                ACTF.Identity,
                                scale=acsb[hb:hb + 64, n:n + 1],
                                bias=acsb[hb:hb + 64, 16 + n:17 + n])
                    of = of2
                # ---- output projection for image b ----
                if stage == 'attn':
                    for p in range(8):
                        nc.sync.dma_start(y_d[0:128, 0:784], of[p][:])
                    continue
                for j in range(JPI):
                    ts = slice(TT * j, TT * (j + 1))
                    for nn in range(2):
                        yp = pqp.tile([TT, 512], F32, tag="pq")
                        for k in range(8):
                            nc.tensor.matmul(yp[:], of[k][:, ts],
                                             wo_sb[k][:, 512 * nn:512 * (nn + 1)],
                                             start=(k == 0), stop=(k == 7))
                        y_sb = youtp.tile([TT, 512], BF16, tag="y_sb")
                        nc.vector.tensor_copy(y_sb[:], yp[:])
                        nc.sync.dma_start(
                            y_d[TPI * b + TT * j:TPI * b + TT * (j + 1),
                                512 * nn:512 * (nn + 1)], y_sb[:])
    nc.compile()
    return nc


def _host_tables():
    inv_freq = 1.0 / (10000.0 ** (np.arange(0, 16, dtype=np.float64) * 2 / 32))
    wpos = np.arange(W, dtype=np.float64)
    ang = wpos[:, None] * inv_freq[None, :]          # [28, 16]
    cosw = np.cos(ang).astype(np.float32)
    sinw = np.sin(ang).astype(np.float32)
    # per-token (within image) tables, replicated per head:
    # C block = [cos, cos]; S block = [-sin, +sin]
    cblk = np.concatenate([cosw, cosw], axis=1)       # [28, 32]
    sblk = np.concatenate([-sinw, sinw], axis=1)      # [28, 32]
    crow = np.tile(cblk, (1, HEADS))                  # [28, 512]
    srow = np.tile(sblk, (1, HEADS))
    ctab = np.tile(crow, (H, 1)).reshape(TPI, 512)    # rows t=r*28+w -> w pattern
    stab = np.tile(srow, (H, 1)).reshape(TPI, 512)
    at = np.zeros((TOK, 128), dtype=np.float32)
    t = np.arange(TOK)
    at[t, 32 * (t // TPI) + (t % W)] = 1.0
    idn = np.eye(128, dtype=np.float32)
    return (ctab.astype(NPBF), stab.astype(NPBF), at.astype(NPBF),
            idn.astype(NPBF))


def _make_in_maps(x, input_img, qkv_w, o_w):
    x = np.ascontiguousarray(np.asarray(x, dtype=np.float32)).astype(NPBF)
    input_img = np.ascontiguousarray(
        np.asarray(input_img, dtype=np.float32)).astype(NPBF)
    qkv_w = np.asarray(qkv_w, dtype=np.float32)
    o_w = np.ascontiguousarray(np.asarray(o_w, dtype=np.float32)).astype(NPBF)
    ctab, stab, at, idn = _host_tables()
    wqk = np.ascontiguousarray(
        np.concatenate([qkv_w[:, 0:HID], qkv_w[:, 2 * HID:3 * HID]],
                       axis=1)).astype(NPBF)
    wv = np.ascontiguousarray(qkv_w[:, HID:2 * HID]).astype(NPBF)

    in_maps = []
    for c in range(N_CORES):
        in_maps.append({
            "x": x[B_CORE * c:B_CORE * (c + 1)].reshape(TOK, HID),
            "img": input_img[B_CORE * c:B_CORE * (c + 1)].reshape(TOK, HID),
            "wqk": wqk, "wv": wv, "wo": o_w,
            "at": at, "idn": idn, "ctab": ctab, "stab": stab,
        })
    return in_maps


def kernel(x, input_img, qkv_w, o_w, gn_w, gn_b):
    gn_w = np.asarray(gn_w, dtype=np.float32)
    gn_b = np.asarray(gn_b, dtype=np.float32)

    key = (tuple(gn_w.tolist()), tuple(gn_b.tolist()))
    if key not in _CACHE:
        _CACHE[key] = _build_program(gn_w, gn_b)
    nc = _CACHE[key]

    in_maps = _make_in_maps(x, input_img, qkv_w, o_w)
    res = run_bass_kernel_spmd(nc, in_maps, list(range(N_CORES)))
    out = np.concatenate(
        [np.asarray(res.results[c]["y"]).astype(np.float32).reshape(
            B_CORE, H, W, HID) for c in range(N_CORES)], axis=0)
    return out


# revision 7
# speedup vs baseline: 3.6159x; 1.5180x over previous
"""AxialAttention Trainium2 kernel (8-core data-parallel over batch).

Per image: qkv = x @ qkv_w + alpha*img; per head (16, dh=64) axial-roped
q,k; scores along W per row (no softmax); v row-summed; GroupNorm per
(b, head); output projection.

Algebraic simplifications (exact to ~1e-9 rel):
  - per-head gamma scale on k is removed by GroupNorm -> dropped.
  - height-half rope rotations cancel in q.k (same row, orthogonal) ->
    rope only on width-half features (32 of 64 per head).
  - v only needed row-summed: vsum = (A @ x) @ Wv + alpha*(A @ img) ->
    the per-token v projection is skipped entirely.

bf16 matmuls (1 cycle/row), fp32 PSUM. x arrives feature-major via XBAR
DMA transpose; x row-sums via DVE reduce; img row-sums + Wv matmuls in
packed [64,512] PSUM chains. Per-image software pipeline
S1(0) S1(1) A(0) S1(2) P(0) A(1) ... hides the GroupNorm stats latency.
"""

import math
import sys

import numpy as np
import ml_dtypes

for _p in ("/opt/trn_rl_repo", "/root/.axon_site/_ro/trn_rl_repo"):
    if _p not in sys.path:
        sys.path.append(_p)

import concourse.bacc as bacc
import concourse.mybir as mybir
from concourse import bass_isa, tile
from concourse.bass_utils import run_bass_kernel_spmd

F32 = mybir.dt.float32
BF16 = mybir.dt.bfloat16
ALU = mybir.AluOpType
ACTF = mybir.ActivationFunctionType
AXL = mybir.AxisListType
NPBF = ml_dtypes.bfloat16

HEADS = 16
DH = 64
H = W = 28
HID = 1024
B_FULL = 32
N_CORES = 8
B_CORE = B_FULL // N_CORES          # 4 images per core
TOK = B_CORE * H * W                # 3136 tokens per core
TT = 112                            # tokens per tile (4 rows)
TPI = H * W                         # 784 tokens per image
JPI = TPI // TT                     # 7 tiles per image
ALPHA = 1.0 - math.tanh(math.pi * 6.0 / 12.0)
EPS = 1e-5
NGRP = float(H * W * DH)

_CACHE = {}


def _build_program(gn_w, gn_b):
    nc = bacc.Bacc("TRN2", target_bir_lowering=False, debug=False,
                   num_devices=N_CORES)

    x_d = nc.dram_tensor("x", [TOK, HID], BF16, kind="ExternalInput").ap()
    img_d = nc.dram_tensor("img", [TOK, HID], BF16, kind="ExternalInput").ap()
    wqk_d = nc.dram_tensor("wqk", [HID, 2 * HID], BF16, kind="ExternalInput").ap()
    wv_d = nc.dram_tensor("wv", [HID, HID], BF16, kind="ExternalInput").ap()
    wo_d = nc.dram_tensor("wo", [HID, HID], BF16, kind="ExternalInput").ap()
    at_d = nc.dram_tensor("at", [TT, 28], BF16, kind="ExternalInput").ap()
    idn_d = nc.dram_tensor("idn", [128, 128], BF16, kind="ExternalInput").ap()
    ct_d = nc.dram_tensor("ctab", [TPI, 512], BF16, kind="ExternalInput").ap()
    st_d = nc.dram_tensor("stab", [TPI, 512], BF16, kind="ExternalInput").ap()
    y_d = nc.dram_tensor("y", [TOK, HID], BF16, kind="ExternalOutput").ap()

    from contextlib import ExitStack
    with ExitStack() as ctx:
        tc = ctx.enter_context(tile.TileContext(nc))
        constp = ctx.enter_context(tc.tile_pool(name="const", bufs=1))
        wqkp = ctx.enter_context(tc.tile_pool(name="wqk", bufs=1))
        wop = ctx.enter_context(tc.tile_pool(name="wo", bufs=1))
        wvp = ctx.enter_context(tc.tile_pool(name="wv", bufs=1))
        tabp = ctx.enter_context(tc.tile_pool(name="tab", bufs=1))
        xtsp = ctx.enter_context(tc.tile_pool(name="xts", bufs=2))
        xsTp = ctx.enter_context(tc.tile_pool(name="xsT", bufs=2))
        imgp = ctx.enter_context(tc.tile_pool(name="imgin", bufs=3))
        qkcp = ctx.enter_context(tc.tile_pool(name="qkc", bufs=2))
        rtp = ctx.enter_context(tc.tile_pool(name="rt", bufs=1))
        qkfp = ctx.enter_context(tc.tile_pool(name="qkf", bufs=32))
        vsp = ctx.enter_context(tc.tile_pool(name="vs", bufs=2))
        stsbp = ctx.enter_context(tc.tile_pool(name="stsb", bufs=2))
        ofpl = ctx.enter_context(tc.tile_pool(name="ofp", bufs=18))
        accp = ctx.enter_context(tc.tile_pool(name="acc", bufs=2))
        youtp = ctx.enter_context(tc.tile_pool(name="yout", bufs=2))
        pqp = ctx.enter_context(tc.tile_pool(name="pq", bufs=2, space="PSUM"))
        rvp = ctx.enter_context(tc.tile_pool(name="rv", bufs=1, space="PSUM"))
        trp = ctx.enter_context(tc.tile_pool(name="tr", bufs=2, space="PSUM"))
        attp = ctx.enter_context(tc.tile_pool(name="att", bufs=3, space="PSUM"))

        # ---------------- preload ----------------
        idn = constp.tile([128, 128], BF16, tag="idn")
        nc.sync.dma_start(idn[:], idn_d[:])
        at_sb = constp.tile([TT, 28], BF16, tag="at")
        nc.sync.dma_start(at_sb[:], at_d[:])
        gw = constp.tile([128, 32], F32, tag="gw")
        epsb = constp.tile([128, 1], F32, tag="epsb")
        nc.gpsimd.memset(epsb[:], EPS)
        for n in range(HEADS):
            nc.gpsimd.memset(gw[:, n:n + 1], float(gn_w[n]))
            nc.gpsimd.memset(gw[:, 16 + n:17 + n], float(gn_b[n]))

        wqk_sb = []
        for k in range(8):
            t = wqkp.tile([128, 2 * HID], BF16, tag=f"wqk{k}", name=f"wqk_sb{k}")
            nc.sync.dma_start(t[:], wqk_d[128 * k:128 * (k + 1), :])
            wqk_sb.append(t)
        wo_sb = []
        for k in range(8):
            t = wop.tile([128, HID], BF16, tag=f"wo{k}", name=f"wo_sb{k}")
            nc.sync.dma_start(t[:], wo_d[128 * k:128 * (k + 1), :])
            wo_sb.append(t)
        wv_sb = []
        for k in range(8):
            t = wvp.tile([128, HID], BF16, tag=f"wv{k}", name=f"wv_sb{k}")
            nc.sync.dma_start(t[:], wv_d[128 * k:128 * (k + 1), :])
            wv_sb.append(t)
        cts, sts = [], []
        for j in range(JPI):
            ct = tabp.tile([TT, 512], BF16, tag=f"ct{j}", name=f"ct{j}")
            nc.sync.dma_start(ct[:], ct_d[TT * j:TT * (j + 1), :])
            st = tabp.tile([TT, 512], BF16, tag=f"st{j}", name=f"st{j}")
            nc.sync.dma_start(st[:], st_d[TT * j:TT * (j + 1), :])
            cts.append(ct)
            sts.append(st)

        vsums = {}   # b -> vsum_sb tile [64, 512] packed
        qfs = {}     # b -> list of 8 qf tiles
        kfs = {}
        ofs = {}     # b -> list of 16 (of tiles per (p,hn) half rows)

        def emit_S1(b):
            rs_img = slice(TPI * b, TPI * (b + 1))
            # x image b, feature-major via XBAR transpose DMA
            xts = []
            for k in range(8):
                t = xtsp.tile([128, TPI], BF16, tag=f"xts{k}",
                              name=f"xts{b}_{k}")
                nc.sync.dma_start(t[:], x_d[rs_img, 128 * k:128 * (k + 1)],
                                  transpose=True)
                xts.append(t)
            # x row-sums (over H) -> feature-major [128, 28] per chunk
            xsT_f = xsTp.tile([128, 8 * 28], F32, tag="xsTf",
                              name=f"xsTf{b}")
            for k in range(8):
                nc.vector.tensor_reduce(
                    xsT_f[:, 28 * k:28 * (k + 1)],
                    xts[k][:].rearrange("p (r w) -> p w r", w=W),
                    axis=AXL.X, op=ALU.add)
            xsT_b = xsTp.tile([128, 8 * 28], BF16, tag="xsTb",
                              name=f"xsTb{b}")
            nc.vector.tensor_copy(xsT_b[:], xsT_f[:])

            qf = [qkfp.tile([128, TPI], BF16, tag="qkf", name=f"qf{b}_{i}")
                  for i in range(8)]
            kf = [qkfp.tile([128, TPI], BF16, tag="qkf", name=f"kf{b}_{i}")
                  for i in range(8)]
            qfs[b], kfs[b] = qf, kf

            # vsum = (A@x)@Wv + alpha*(A@img), ONE psum chain packed [64,512]:
            # rows 0:28 = feature cols 0:512, rows 32:60 = cols 512:1024.
            # alpha is folded into the `at` one-hot matrix host-side; the Wv
            # matmuls open the chain, the per-tile A@img matmuls close it.
            vch = rvp.tile([64, 512], F32, tag="rv", name=f"vch{b}")
            for k in range(8):
                nc.tensor.matmul(vch[0:28, :],
                                 xsT_b[:, 28 * k:28 * (k + 1)],
                                 wv_sb[k][:, 0:512],
                                 tile_position=(0, 0),
                                 start=(k == 0), stop=False)
                nc.tensor.matmul(vch[32:60, :],
                                 xsT_b[:, 28 * k:28 * (k + 1)],
                                 wv_sb[k][:, 512:1024],
                                 tile_position=(0, 32),
                                 start=(k == 0), stop=False)
            for j in range(JPI):
                i = JPI * b + j
                rs = slice(TT * i, TT * (i + 1))
                it1 = imgp.tile([TT, HID], BF16, tag="i0")
                nc.sync.dma_start(it1[:], img_d[rs, :])
                nc.tensor.matmul(vch[0:28, :], at_sb[:], it1[:, 0:512],
                                 tile_position=(0, 0),
                                 start=False, stop=(j == JPI - 1))
                nc.tensor.matmul(vch[32:60, :], at_sb[:], it1[:, 512:1024],
                                 tile_position=(0, 32),
                                 start=False, stop=(j == JPI - 1))
                ct, st = cts[j], sts[j]
                ts = slice(TT * j, TT * (j + 1))
                for n in range(4):   # 512-col chunks of [q | k]
                    pq = pqp.tile([TT, 512], F32, tag="pq")
                    for k in range(8):
                        nc.tensor.matmul(pq[:], xts[k][:, ts],
                                         wqk_sb[k][:, 512 * n:512 * (n + 1)],
                                         start=(k == 0), stop=(k == 7))
                    qkc = qkcp.tile([TT, 512], BF16, tag="qkc")
                    nc.vector.scalar_tensor_tensor(
                        qkc[:], it1[:, 512 * (n % 2):512 * (n % 2 + 1)],
                        ALPHA, pq[:], ALU.mult, ALU.add)
                    # rope on width-halves (8 heads per chunk)
                    hh = 8 * (n % 2)
                    qv = qkc[:].rearrange("p (h d) -> p h d", d=64)[:, :, 32:64]
                    cv = ct[:].rearrange("p (h d) -> p h d", d=32)[:, hh:hh + 8, :]
                    sv = st[:].rearrange("p (h d) -> p h d", d=32)[:, hh:hh + 8, :]
                    t1 = rtp.tile([TT, 256], BF16, tag="t1")
                    t1v = t1[:].rearrange("p (h d) -> p h d", d=32)
                    t2 = rtp.tile([TT, 256], BF16, tag="t2")
                    t2v = t2[:].rearrange("p (h d) -> p h d", d=32)
                    nc.vector.tensor_tensor(t1v[:], qv[:], cv[:], op=ALU.mult)
                    nc.vector.tensor_tensor(t2v[:, :, 0:16], qv[:, :, 16:32],
                                            sv[:, :, 0:16], op=ALU.mult)
                    nc.vector.tensor_tensor(t2v[:, :, 16:32], qv[:, :, 0:16],
                                            sv[:, :, 16:32], op=ALU.mult)
                    nc.vector.tensor_tensor(qv[:], t1v[:], t2v[:], op=ALU.add)
                    for c in range(4):
                        cg = 4 * n + c
                        tp = trp.tile([128, TT], BF16, tag="tr")
                        nc.tensor.transpose(tp[:],
                                            qkc[:, 128 * c:128 * (c + 1)],
                                            idn[0:TT, 0:TT])
                        dst = qf[cg] if cg < 8 else kf[cg - 8]
                        if c % 2 == 0:
                            nc.scalar.copy(dst[:, ts], tp[:])
                        else:
                            nc.vector.tensor_copy(dst[:, ts], tp[:])

            vsum = vsp.tile([64, 512], BF16, tag="vs", name=f"vsum{b}")
            nc.vector.tensor_copy(vsum[0:60, :], vch[0:60, :])
            vsums[b] = vsum

        def emit_A(b):
            qf, kf, vsum = qfs[b], kfs[b], vsums[b]
            statb = accp.tile([128, 64], F32, tag="statb", name=f"statb{b}")
            nc.gpsimd.memset(statb[:], 0.0)
            of = []
            for p in range(8):
                rb = 0 if p < 4 else 32    # vsum/score partition row base
                ofp = ofpl.tile([128, TPI], BF16, tag="of", name=f"of{b}_{p}")
                of.append(ofp)
                for hn in range(2):
                    n = 2 * p + hn
                    hb = 64 * hn
                    stp_a = attp.tile([128, 448], F32, tag="att",
                                      name=f"stpa{b}_{p}_{hn}")
                    stp_b = attp.tile([128, 448], F32, tag="att",
                                      name=f"stpb{b}_{p}_{hn}")
                    for r in range(28):
                        dst = stp_a if r < 14 else stp_b
                        rr = r % 14
                        nc.tensor.matmul(
                            dst[rb:rb + 28, 32 * rr:32 * rr + 28],
                            kf[p][hb:hb + 64, 28 * r:28 * (r + 1)],
                            qf[p][hb:hb + 64, 28 * r:28 * (r + 1)],
                            tile_position=(hb, rb),
                            start=True, stop=True)
                    st_sb = stsbp.tile([128, TPI], BF16, tag="st_sb")
                    for half, stp in ((0, stp_a), (1, stp_b)):
                        stv = stp[rb:rb + 28, :].rearrange(
                            "p (r c) -> p r c", c=32)[:, :, 0:28]
                        dstv = st_sb[rb:rb + 28,
                                     392 * half:392 * (half + 1)].rearrange(
                            "p (r c) -> p r c", c=28)
                        nc.vector.tensor_copy(dstv, stv)
                    # out = vsum_n^T @ S^T  (feature-major of), [64, 784]
                    vcol = 64 * (n if p < 4 else n - 8)
                    otp_a = attp.tile([128, 512], F32, tag="att",
                                      name=f"otpa{b}_{p}_{hn}")
                    otp_b = attp.tile([128, 272], F32, tag="att",
                                      name=f"otpb{b}_{p}_{hn}")
                    nc.tensor.matmul(otp_a[hb:hb + 64, :],
                                     vsum[rb:rb + 28, vcol:vcol + 64],
                                     st_sb[rb:rb + 28, 0:512],
                                     tile_position=(rb, hb),
                                     start=True, stop=True)
                    nc.tensor.matmul(otp_b[hb:hb + 64, :],
                                     vsum[rb:rb + 28, vcol:vcol + 64],
                                     st_sb[rb:rb + 28, 512:TPI],
                                     tile_position=(rb, hb),
                                     start=True, stop=True)
                    nc.scalar.activation(ofp[hb:hb + 64, 0:512],
                                         otp_a[hb:hb + 64, :], ACTF.Copy,
                                         accum_out=statb[hb:hb + 64, n:n + 1])
                    nc.scalar.activation(ofp[hb:hb + 64, 512:TPI],
                                         otp_b[hb:hb + 64, :], ACTF.Copy,
                                         accum_out=statb[hb:hb + 64,
                                                         32 + n:33 + n])
                    sqt = ofpl.tile([128, TPI], BF16, tag="sq", bufs=2,
                                    name=f"sq{b}_{p}_{hn}")
                    nc.scalar.activation(sqt[hb:hb + 64, :],
                                         ofp[hb:hb + 64, :], ACTF.Square,
                                         accum_out=statb[hb:hb + 64,
                                                         16 + n:17 + n])

            allred = accp.tile([128, 64], F32, tag="allred", name=f"ar{b}")
            nc.gpsimd.partition_all_reduce(
                allred[:], statb[:], channels=128,
                reduce_op=bass_isa.ReduceOp.add)
            sum_ab = accp.tile([128, 16], F32, tag="sum_ab", name=f"sab{b}")
            nc.vector.tensor_tensor(sum_ab[:], allred[:, 0:16],
                                    allred[:, 32:48], op=ALU.add)
            m2 = accp.tile([128, 32], F32, tag="m2", name=f"m2{b}")
            nc.scalar.mul(m2[:, 0:16], sum_ab[:], 1.0 / NGRP)
            nc.scalar.mul(m2[:, 16:32], allred[:, 16:32], 1.0 / NGRP)
            msq = accp.tile([128, 16], F32, tag="msq", name=f"msq{b}")
            nc.scalar.activation(msq[:], m2[:, 0:16], ACTF.Square)
            var = accp.tile([128, 16], F32, tag="var", name=f"var{b}")
            nc.vector.tensor_tensor(var[:], m2[:, 16:32], msq[:],
                                    op=ALU.subtract)
            sd = accp.tile([128, 16], F32, tag="sd", name=f"sd{b}")
            nc.scalar.activation(sd[:], var[:], ACTF.Sqrt, bias=epsb[:, 0:1])
            inv = accp.tile([128, 16], F32, tag="inv", name=f"inv{b}")
            nc.vector.reciprocal(inv[:], sd[:])
            acsb = accp.tile([128, 32], F32, tag="acsb", name=f"acsb{b}")
            nc.vector.tensor_tensor(acsb[:, 0:16], inv[:], gw[:, 0:16],
                                    op=ALU.mult)
            ctmp = accp.tile([128, 16], F32, tag="ctmp", name=f"ctmp{b}")
            nc.vector.scalar_tensor_tensor(ctmp[:], m2[:, 0:16], -1.0,
                                           acsb[:, 0:16], ALU.mult, ALU.mult)
            nc.vector.tensor_tensor(acsb[:, 16:32], ctmp[:], gw[:, 16:32],
                                    op=ALU.add)
            of2 = []
            for p in range(8):
                of2p = ofpl.tile([128, TPI], BF16, tag="of",
                                 name=f"of2_{b}_{p}")
                of2.append(of2p)
                for hn in range(2):
                    n = 2 * p + hn
                    hb = 64 * hn
                    nc.scalar.activation(
                        of2p[hb:hb + 64, :], of[p][hb:hb + 64, :],
                        ACTF.Identity,
                        scale=acsb[hb:hb + 64, n:n + 1],
                        bias=acsb[hb:hb + 64, 16 + n:17 + n])
            ofs[b] = of2

        def emit_P(b):
            of2 = ofs[b]
            for j in range(JPI):
                ts = slice(TT * j, TT * (j + 1))
                for nn in range(2):
                    yp = pqp.tile([TT, 512], F32, tag="pq")
                    for k in range(8):
                        nc.tensor.matmul(yp[:], of2[k][:, ts],
                                         wo_sb[k][:, 512 * nn:512 * (nn + 1)],
                                         start=(k == 0), stop=(k == 7))
                    y_sb = youtp.tile([TT, 512], BF16, tag="y_sb")
                    nc.vector.tensor_copy(y_sb[:], yp[:])
                    nc.sync.dma_start(
                        y_d[TPI * b + TT * j:TPI * b + TT * (j + 1),
                            512 * nn:512 * (nn + 1)], y_sb[:])

        # software pipeline: hide stats latency behind next image's proj
        emit_S1(0)
        emit_S1(1)
        emit_A(0)
        emit_S1(2)
        emit_P(0)
        emit_A(1)
        emit_S1(3)
        emit_P(1)
        emit_A(2)
        emit_P(2)
        emit_A(3)
        emit_P(3)
    nc.compile()
    return nc


def _host_tables():
    inv_freq = 1.0 / (10000.0 ** (np.arange(0, 16, dtype=np.float64) * 2 / 32))
    wpos = np.arange(W, dtype=np.float64)
    ang = wpos[:, None] * inv_freq[None, :]          # [28, 16]
    cosw = np.cos(ang).astype(np.float32)
    sinw = np.sin(ang).astype(np.float32)
    # per-token (within image) tables, replicated per head:
    # C block = [cos, cos]; S block = [-sin, +sin]
    cblk = np.concatenate([cosw, cosw], axis=1)       # [28, 32]
    sblk = np.concatenate([-sinw, sinw], axis=1)      # [28, 32]
    crow = np.tile(cblk, (1, HEADS))                  # [28, 512]
    srow = np.tile(sblk, (1, HEADS))
    ctab = np.tile(crow, (H, 1)).reshape(TPI, 512)    # rows t=r*28+w -> w pattern
    stab = np.tile(srow, (H, 1)).reshape(TPI, 512)
    at = np.zeros((TT, 28), dtype=np.float32)
    p = np.arange(TT)
    at[p, p % W] = ALPHA    # alpha folded into the img row-sum one-hot
    idn = np.eye(128, dtype=np.float32)
    return (ctab.astype(NPBF), stab.astype(NPBF), at.astype(NPBF),
            idn.astype(NPBF))


def _make_in_maps(x, input_img, qkv_w, o_w):
    x = np.ascontiguousarray(np.asarray(x, dtype=np.float32)).astype(NPBF)
    input_img = np.ascontiguousarray(
        np.asarray(input_img, dtype=np.float32)).astype(NPBF)
    qkv_w = np.asarray(qkv_w, dtype=np.float32)
    o_w = np.ascontiguousarray(np.asarray(o_w, dtype=np.float32)).astype(NPBF)
    ctab, stab, at, idn = _host_tables()
    wqk = np.ascontiguousarray(
        np.concatenate([qkv_w[:, 0:HID], qkv_w[:, 2 * HID:3 * HID]],
                       axis=1)).astype(NPBF)
    wv = np.ascontiguousarray(qkv_w[:, HID:2 * HID]).astype(NPBF)

    in_maps = []
    for c in range(N_CORES):
        in_maps.append({
            "x": x[B_CORE * c:B_CORE * (c + 1)].reshape(TOK, HID),
            "img": input_img[B_CORE * c:B_CORE * (c + 1)].reshape(TOK, HID),
            "wqk": wqk, "wv": wv, "wo": o_w,
            "at": at, "idn": idn, "ctab": ctab, "stab": stab,
        })
    return in_maps


def kernel(x, input_img, qkv_w, o_w, gn_w, gn_b):
    gn_w = np.asarray(gn_w, dtype=np.float32)
    gn_b = np.asarray(gn_b, dtype=np.float32)

    key = (tuple(gn_w.tolist()), tuple(gn_b.tolist()))
    if key not in _CACHE:
        _CACHE[key] = _build_program(gn_w, gn_b)
    nc = _CACHE[key]

    in_maps = _make_in_maps(x, input_img, qkv_w, o_w)
    res = run_bass_kernel_spmd(nc, in_maps, list(range(N_CORES)))
    out = np.concatenate(
        [np.asarray(res.results[c]["y"]).astype(np.float32).reshape(
            B_CORE, H, W, HID) for c in range(N_CORES)], axis=0)
    return out


# revision 27
# speedup vs baseline: 3.8264x; 1.0582x over previous
"""AxialAttention Trainium2 kernel (8-core data-parallel over batch).

Per image: qkv = x @ qkv_w + alpha*img; per head (16, dh=64) axial-roped
q,k; scores along W per row (no softmax); v row-summed; GroupNorm per
(b, head); output projection.

Algebraic simplifications (exact to ~1e-9 rel):
  - per-head gamma scale on k is removed by GroupNorm -> dropped.
  - height-half rope rotations cancel in q.k (same row, orthogonal) ->
    rope only on width-half features (32 of 64 per head).
  - v only needed row-summed: vsum = (A @ x) @ Wv + alpha*(A @ img) ->
    the per-token v projection is skipped entirely.

bf16 matmuls (1 cycle/row), fp32 PSUM. x arrives feature-major via XBAR
DMA transpose; x row-sums via DVE reduce; img row-sums + Wv matmuls in
packed [64,512] PSUM chains. Per-image software pipeline
S1(0) S1(1) A(0) S1(2) P(0) A(1) ... hides the GroupNorm stats latency.
"""

import math
import sys

import numpy as np
import ml_dtypes

for _p in ("/opt/trn_rl_repo", "/root/.axon_site/_ro/trn_rl_repo"):
    if _p not in sys.path:
        sys.path.append(_p)

import concourse.bacc as bacc
import concourse.mybir as mybir
from concourse import bass_isa, tile
from concourse.bass_utils import run_bass_kernel_spmd

F32 = mybir.dt.float32
BF16 = mybir.dt.bfloat16
ALU = mybir.AluOpType
ACTF = mybir.ActivationFunctionType
AXL = mybir.AxisListType
NPBF = ml_dtypes.bfloat16

HEADS = 16
DH = 64
H = W = 28
HID = 1024
B_FULL = 32
N_CORES = 8
B_CORE = B_FULL // N_CORES          # 4 images per core
TOK = B_CORE * H * W                # 3136 tokens per core
TT = 112                            # tokens per tile (4 rows)
TPI = H * W                         # 784 tokens per image
JPI = TPI // TT                     # 7 tiles per image
ALPHA = 1.0 - math.tanh(math.pi * 6.0 / 12.0)
EPS = 1e-5
NGRP = float(H * W * DH)

_CACHE = {}


def _build_program(gn_w, gn_b):
    nc = bacc.Bacc("TRN2", target_bir_lowering=False, debug=False,
                   num_devices=N_CORES)

    x_d = nc.dram_tensor("x", [TOK, HID], BF16, kind="ExternalInput").ap()
    img_d = nc.dram_tensor("img", [TOK, HID], BF16, kind="ExternalInput").ap()
    wqk_d = nc.dram_tensor("wqk", [HID, 2 * HID], BF16, kind="ExternalInput").ap()
    wv_d = nc.dram_tensor("wv", [HID, HID], BF16, kind="ExternalInput").ap()
    wo_d = nc.dram_tensor("wo", [HID, HID], BF16, kind="ExternalInput").ap()
    at_d = nc.dram_tensor("at", [TT, 28], BF16, kind="ExternalInput").ap()
    idn_d = nc.dram_tensor("idn", [128, 128], BF16, kind="ExternalInput").ap()
    ct_d = nc.dram_tensor("ctab", [TPI, 512], BF16, kind="ExternalInput").ap()
    st_d = nc.dram_tensor("stab", [TPI, 512], BF16, kind="ExternalInput").ap()
    y_d = nc.dram_tensor("y", [TOK, HID], BF16, kind="ExternalOutput").ap()

    from contextlib import ExitStack
    with ExitStack() as ctx:
        tc = ctx.enter_context(tile.TileContext(nc))
        constp = ctx.enter_context(tc.tile_pool(name="const", bufs=1))
        wqkp = ctx.enter_context(tc.tile_pool(name="wqk", bufs=1))
        wop = ctx.enter_context(tc.tile_pool(name="wo", bufs=1))
        wvp = ctx.enter_context(tc.tile_pool(name="wv", bufs=1))
        tabp = ctx.enter_context(tc.tile_pool(name="tab", bufs=1))
        xtsp = ctx.enter_context(tc.tile_pool(name="xts", bufs=2))
        xsTp = ctx.enter_context(tc.tile_pool(name="xsT", bufs=1))
        imgp = ctx.enter_context(tc.tile_pool(name="imgin", bufs=2))
        qkcp = ctx.enter_context(tc.tile_pool(name="qkc", bufs=2))
        rtp = ctx.enter_context(tc.tile_pool(name="rt", bufs=1))
        qkfp = ctx.enter_context(tc.tile_pool(name="qkf", bufs=4))
        isap = ctx.enter_context(tc.tile_pool(name="isa", bufs=2))
        vsp = ctx.enter_context(tc.tile_pool(name="vs", bufs=2))
        stsbp = ctx.enter_context(tc.tile_pool(name="stsb", bufs=2))
        ofpl = ctx.enter_context(tc.tile_pool(name="ofp", bufs=16))
        accp = ctx.enter_context(tc.tile_pool(name="acc", bufs=2))
        youtp = ctx.enter_context(tc.tile_pool(name="yout", bufs=2))
        pqp = ctx.enter_context(tc.tile_pool(name="pq", bufs=2, space="PSUM"))
        trp = ctx.enter_context(tc.tile_pool(name="tr", bufs=2, space="PSUM"))
        stpp = ctx.enter_context(tc.tile_pool(name="stp", bufs=2, space="PSUM"))
        otpp = ctx.enter_context(tc.tile_pool(name="otp", bufs=1, space="PSUM"))

        # ---------------- preload (wqk + x(0) first for fast start) ------
        idn = constp.tile([128, 128], BF16, tag="idn")
        nc.sync.dma_start(idn[:], idn_d[:])
        at_sb = constp.tile([TT, 28], BF16, tag="at")
        nc.sync.dma_start(at_sb[:], at_d[:])
        # gw packed per head-pair: col p rows 0:64 = head 2p, rows 64:128 =
        # head 2p+1; cols 0:8 scale, 8:16 bias
        gw = constp.tile([128, 16], F32, tag="gw")
        epsb = constp.tile([128, 1], F32, tag="epsb")
        nc.gpsimd.memset(epsb[:], EPS)
        # block-diag ones: sums each 64-partition half and redistributes
        hred = constp.tile([128, 128], F32, tag="hred")
        nc.gpsimd.memset(hred[0:64, 0:64], 1.0)
        nc.gpsimd.memset(hred[0:64, 64:128], 0.0)
        nc.gpsimd.memset(hred[64:128, 0:64], 0.0)
        nc.gpsimd.memset(hred[64:128, 64:128], 1.0)
        for p in range(8):
            nc.gpsimd.memset(gw[0:64, p:p + 1], float(gn_w[2 * p]))
            nc.gpsimd.memset(gw[64:128, p:p + 1], float(gn_w[2 * p + 1]))
            nc.gpsimd.memset(gw[0:64, 8 + p:9 + p], float(gn_b[2 * p]))
            nc.gpsimd.memset(gw[64:128, 8 + p:9 + p], float(gn_b[2 * p + 1]))

        wqk_sb = []
        for k in range(8):
            t = wqkp.tile([128, 2 * HID], BF16, tag=f"wqk{k}", name=f"wqk_sb{k}")
            nc.sync.dma_start(t[:], wqk_d[128 * k:128 * (k + 1), :])
            wqk_sb.append(t)

        xts_pre = {}

        def prefetch_x(b):
            rs_img = slice(TPI * b, TPI * (b + 1))
            xts = []
            for k in range(8):
                t = xtsp.tile([128, TPI], BF16, tag=f"xts{k}",
                              name=f"xts{b}_{k}")
                nc.sync.dma_start(t[:], x_d[rs_img, 128 * k:128 * (k + 1)],
                                  transpose=True)
                xts.append(t)
            xts_pre[b] = xts

        prefetch_x(0)

        wo_sb = []
        for k in range(8):
            t = wop.tile([128, HID], BF16, tag=f"wo{k}", name=f"wo_sb{k}")
            nc.sync.dma_start(t[:], wo_d[128 * k:128 * (k + 1), :])
            wo_sb.append(t)
        wv_sb = []
        for k in range(8):
            t = wvp.tile([128, HID], BF16, tag=f"wv{k}", name=f"wv_sb{k}")
            nc.sync.dma_start(t[:], wv_d[128 * k:128 * (k + 1), :])
            wv_sb.append(t)
        cts, sts = [], []
        for j in range(JPI):
            ct = tabp.tile([TT, 512], BF16, tag=f"ct{j}", name=f"ct{j}")
            nc.sync.dma_start(ct[:], ct_d[TT * j:TT * (j + 1), :])
            st = tabp.tile([TT, 512], BF16, tag=f"st{j}", name=f"st{j}")
            nc.sync.dma_start(st[:], st_d[TT * j:TT * (j + 1), :])
            cts.append(ct)
            sts.append(st)

        vsums = {}   # b -> vsum_sb tile [64, 512] packed
        qfs = {}     # b -> list of 8 qf tiles
        kfs = {}
        ofs = {}     # b -> list of 8 of2 tiles

        def emit_S1(b):
            xts = xts_pre.pop(b)
            # qf/kf: one big tile each, head-pair p at cols 784p..784(p+1)
            qf = qkfp.tile([128, 8 * TPI], BF16, tag="qkf", name=f"qf{b}")
            kf = qkfp.tile([128, 8 * TPI], BF16, tag="qkf", name=f"kf{b}")
            qfs[b], kfs[b] = qf, kf
            qfv = qf[:].rearrange("p (c t) -> p c t", t=TPI)
            kfv = kf[:].rearrange("p (c t) -> p c t", t=TPI)

            # alpha*(A@img) accumulated in SBUF, packed [64, 512]:
            # rows 0:28 = feature cols 0:512, rows 32:60 = cols 512:1024.
            # alpha is folded into the `at` one-hot matrix host-side.
            is_acc = isap.tile([64, 512], F32, tag="isacc", name=f"isacc{b}")
            for j in range(JPI):
                i = JPI * b + j
                rs = slice(TT * i, TT * (i + 1))
                it1 = imgp.tile([TT, HID], BF16, tag="i0")
                nc.sync.dma_start(it1[:], img_d[rs, :])
                atps = trp.tile([64, 512], F32, tag="tr", name=f"atps{b}_{j}")
                nc.tensor.matmul(atps[0:28, :], at_sb[:], it1[:, 0:512],
                                 tile_position=(0, 0), start=True, stop=True,
                                 skip_group_check=True)
                nc.tensor.matmul(atps[32:60, :], at_sb[:], it1[:, 512:1024],
                                 tile_position=(0, 32), start=True, stop=True,
                                 skip_group_check=True)
                for rr in (slice(0, 28), slice(32, 60)):
                    if j == 0:
                        nc.vector.tensor_copy(is_acc[rr, :], atps[rr, :])
                    else:
                        nc.vector.tensor_tensor(is_acc[rr, :], is_acc[rr, :],
                                                atps[rr, :], op=ALU.add)
                ct, st = cts[j], sts[j]
                ts = slice(TT * j, TT * (j + 1))
                qkc = qkcp.tile([TT, 2 * HID], BF16, tag="qkc")
                for n in range(4):   # 512-col chunks of [q | k]
                    pq = pqp.tile([TT, 512], F32, tag="pq")
                    for k in range(8):
                        nc.tensor.matmul(pq[:], xts[k][:, ts],
                                         wqk_sb[k][:, 512 * n:512 * (n + 1)],
                                         start=(k == 0), stop=(k == 7))
                    nc.vector.scalar_tensor_tensor(
                        qkc[:, 512 * n:512 * (n + 1)],
                        it1[:, 512 * (n % 2):512 * (n % 2 + 1)],
                        ALPHA, pq[:], ALU.mult, ALU.add)
                # rope on width-halves, one [q | k] group at a time (16 heads)
                cv = ct[:].rearrange("p (h d) -> p h d", d=32)
                sv = st[:].rearrange("p (h d) -> p h d", d=32)
                for g in range(2):
                    qv = qkc[:, HID * g:HID * (g + 1)].rearrange(
                        "p (h d) -> p h d", d=64)[:, :, 32:64]
                    t1 = rtp.tile([TT, 512], BF16, tag="t1")
                    t1v = t1[:].rearrange("p (h d) -> p h d", d=32)
                    t2 = rtp.tile([TT, 512], BF16, tag="t2")
                    t2v = t2[:].rearrange("p (h d) -> p h d", d=32)
                    nc.vector.tensor_tensor(t1v[:], qv[:], cv[:], op=ALU.mult)
                    nc.vector.tensor_tensor(t2v[:, :, 0:16], qv[:, :, 16:32],
                                            sv[:, :, 0:16], op=ALU.mult)
                    nc.vector.tensor_tensor(t2v[:, :, 16:32], qv[:, :, 0:16],
                                            sv[:, :, 16:32], op=ALU.mult)
                    nc.vector.tensor_tensor(qv[:], t1v[:], t2v[:], op=ALU.add)
                # 4 transposes per psum tile, one strided copy each
                for grp in range(4):
                    trt = trp.tile([128, 4 * TT], BF16, tag="tr",
                                   name=f"trt{b}_{j}_{grp}")
                    for c4 in range(4):
                        c = 4 * grp + c4
                        nc.tensor.transpose(trt[:, TT * c4:TT * (c4 + 1)],
                                            qkc[:, 128 * c:128 * (c + 1)],
                                            idn[0:TT, 0:TT])
                    dstv = (qfv if grp < 2 else kfv)[
                        :, 4 * (grp % 2):4 * (grp % 2) + 4, ts]
                    srcv = trt[:].rearrange("p (c t) -> p c t", t=TT)
                    if grp % 2 == 0:
                        nc.scalar.copy(dstv, srcv)
                    else:
                        nc.vector.tensor_copy(dstv, srcv)

            # x row-sums (over H) -> feature-major [128, 28] per chunk
            xsT_f = xsTp.tile([128, 8 * 28], F32, tag="xsTf",
                              name=f"xsTf{b}")
            for k in range(8):
                nc.vector.tensor_reduce(
                    xsT_f[:, 28 * k:28 * (k + 1)],
                    xts[k][:].rearrange("p (r w) -> p w r", w=W),
                    axis=AXL.X, op=ALU.add)
            xsT_b = xsTp.tile([128, 8 * 28], BF16, tag="xsTb",
                              name=f"xsTb{b}")
            nc.vector.tensor_copy(xsT_b[:], xsT_f[:])
            # (A@x)@Wv chain, packed like is_acc
            wvch = trp.tile([64, 512], F32, tag="tr", name=f"wvch{b}")
            for half, rr, tpos in ((0, slice(0, 28), (0, 0)),
                                   (1, slice(32, 60), (0, 32))):
                for k in range(8):
                    nc.tensor.matmul(wvch[rr, :],
                                     xsT_b[:, 28 * k:28 * (k + 1)],
                                     wv_sb[k][:, 512 * half:512 * (half + 1)],
                                     tile_position=tpos,
                                     start=(k == 0), stop=(k == 7),
                                     skip_group_check=True)
            vsum = vsp.tile([64, 512], BF16, tag="vs", name=f"vsum{b}")
            for rr in (slice(0, 28), slice(32, 60)):
                nc.vector.scalar_tensor_tensor(vsum[rr, :], is_acc[rr, :],
                                               1.0, wvch[rr, :],
                                               ALU.mult, ALU.add)
            vsums[b] = vsum
            if b + 1 < B_CORE:
                prefetch_x(b + 1)

        def emit_A(b):
            qf, kf, vsum = qfs[b], kfs[b], vsums[b]
            # statb packed per head-pair p: head 2p on rows 0:64, head 2p+1
            # on rows 64:128. cols p: sum(512-part), 8+p: sum(272-part),
            # 16+p: sum of squares.
            statb = accp.tile([128, 24], F32, tag="statb", name=f"statb{b}")
            nc.gpsimd.memset(statb[:], 0.0)
            of = []
            for p in range(8):
                rb = 0 if p < 4 else 32    # vsum/score partition row base
                pc = TPI * p               # col base in the big qf/kf tiles
                ofp = ofpl.tile([128, TPI], BF16, tag="of", name=f"of{b}_{p}")
                of.append(ofp)
                otp = otpp.tile([128, TPI], F32, tag="otp", name=f"otp{b}_{p}")
                for hn in range(2):
                    n = 2 * p + hn
                    hb = 64 * hn
                    stp_a = stpp.tile([128, 448], F32, tag="stp",
                                      name=f"stpa{b}_{p}_{hn}")
                    stp_b = stpp.tile([128, 448], F32, tag="stp",
                                      name=f"stpb{b}_{p}_{hn}")
                    for r in range(28):
                        dst = stp_a if r < 14 else stp_b
                        rr = r % 14
                        nc.tensor.matmul(
                            dst[rb:rb + 28, 32 * rr:32 * rr + 28],
                            kf[hb:hb + 64, pc + 28 * r:pc + 28 * (r + 1)],
                            qf[hb:hb + 64, pc + 28 * r:pc + 28 * (r + 1)],
                            tile_position=(hb, rb),
                            start=True, stop=True, skip_group_check=True)
                    st_sb = stsbp.tile([128, TPI], BF16, tag="st_sb")
                    for half, stp in ((0, stp_a), (1, stp_b)):
                        stv = stp[rb:rb + 28, :].rearrange(
                            "p (r c) -> p r c", c=32)[:, :, 0:28]
                        dstv = st_sb[rb:rb + 28,
                                     392 * half:392 * (half + 1)].rearrange(
                            "p (r c) -> p r c", c=28)
                        if (p + hn) % 2 == 0:
                            nc.vector.tensor_copy(dstv, stv)
                        else:
                            nc.scalar.copy(dstv, stv)
                    # out = vsum_n^T @ S^T  (feature-major of), [64, 784]
                    vcol = 64 * (n if p < 4 else n - 8)
                    nc.tensor.matmul(otp[hb:hb + 64, 0:512],
                                     vsum[rb:rb + 28, vcol:vcol + 64],
                                     st_sb[rb:rb + 28, 0:512],
                                     tile_position=(rb, hb),
                                     start=True, stop=True,
                                     skip_group_check=True)
                    nc.tensor.matmul(otp[hb:hb + 64, 512:TPI],
                                     vsum[rb:rb + 28, vcol:vcol + 64],
                                     st_sb[rb:rb + 28, 512:TPI],
                                     tile_position=(rb, hb),
                                     start=True, stop=True,
                                     skip_group_check=True)
                # both heads of pair p done -> full-height ACT passes
                nc.scalar.activation(ofp[:, 0:512], otp[:, 0:512], ACTF.Copy,
                                     accum_out=statb[:, p:p + 1])
                nc.scalar.activation(ofp[:, 512:TPI], otp[:, 512:TPI],
                                     ACTF.Copy,
                                     accum_out=statb[:, 8 + p:9 + p])
                sqt = ofpl.tile([128, TPI], BF16, tag="sq", bufs=2,
                                name=f"sq{b}_{p}")
                nc.scalar.activation(sqt[:], ofp[:], ACTF.Square,
                                     accum_out=statb[:, 16 + p:17 + p])

            # stats: sum each 64-partition half (head 2p vs head 2p+1 live on
            # different halves of each statb column) via one fp32 matmul with
            # the block-diag ones matrix; result replicated over each half.
            allred_ps = otpp.tile([128, 24], F32, tag="otp",
                                  name=f"arps{b}")
            nc.tensor.matmul(allred_ps[:], hred[:], statb[:],
                             start=True, stop=True, skip_group_check=True)
            allred = accp.tile([128, 24], F32, tag="allred", name=f"ar{b}")
            nc.vector.tensor_copy(allred[:], allred_ps[:])
            sum_ab = accp.tile([128, 8], F32, tag="sum_ab", name=f"sab{b}")
            nc.vector.tensor_tensor(sum_ab[:], allred[:, 0:8],
                                    allred[:, 8:16], op=ALU.add)
            m2 = accp.tile([128, 16], F32, tag="m2", name=f"m2{b}")
            nc.scalar.mul(m2[:, 0:8], sum_ab[:], 1.0 / NGRP)
            nc.scalar.mul(m2[:, 8:16], allred[:, 16:24], 1.0 / NGRP)
            msq = accp.tile([128, 8], F32, tag="msq", name=f"msq{b}")
            nc.scalar.activation(msq[:], m2[:, 0:8], ACTF.Square)
            var = accp.tile([128, 8], F32, tag="var", name=f"var{b}")
            nc.vector.tensor_tensor(var[:], m2[:, 8:16], msq[:],
                                    op=ALU.subtract)
            sd = accp.tile([128, 8], F32, tag="sd", name=f"sd{b}")
            nc.scalar.activation(sd[:], var[:], ACTF.Sqrt, bias=epsb[:, 0:1])
            inv = accp.tile([128, 8], F32, tag="inv", name=f"inv{b}")
            nc.vector.reciprocal(inv[:], sd[:])
            acsb = accp.tile([128, 16], F32, tag="acsb", name=f"acsb{b}")
            nc.vector.tensor_tensor(acsb[:, 0:8], inv[:], gw[:, 0:8],
                                    op=ALU.mult)
            ctmp = accp.tile([128, 8], F32, tag="ctmp", name=f"ctmp{b}")
            nc.vector.scalar_tensor_tensor(ctmp[:], m2[:, 0:8], -1.0,
                                           acsb[:, 0:8], ALU.mult, ALU.mult)
            nc.vector.tensor_tensor(acsb[:, 8:16], ctmp[:], gw[:, 8:16],
                                    op=ALU.add)
            of2 = []
            for p in range(8):
                of2p = ofpl.tile([128, TPI], BF16, tag="of",
                                 name=f"of2_{b}_{p}")
                of2.append(of2p)
                nc.scalar.activation(
                    of2p[:], of[p][:], ACTF.Identity,
                    scale=acsb[:, p:p + 1],
                    bias=acsb[:, 8 + p:9 + p])
            ofs[b] = of2

        def emit_P(b):
            of2 = ofs[b]
            for j in range(JPI):
                ts = slice(TT * j, TT * (j + 1))
                for nn in range(2):
                    yp = pqp.tile([TT, 512], F32, tag="pq")
                    for k in range(8):
                        nc.tensor.matmul(yp[:], of2[k][:, ts],
                                         wo_sb[k][:, 512 * nn:512 * (nn + 1)],
                                         start=(k == 0), stop=(k == 7))
                    y_sb = youtp.tile([TT, 512], BF16, tag="y_sb")
                    nc.vector.tensor_copy(y_sb[:], yp[:])
                    # scalar hwdge queue: keeps y stores off the sync queue
                    # (no head-of-line blocking of img/x loads)
                    nc.scalar.dma_start(
                        y_d[TPI * b + TT * j:TPI * b + TT * (j + 1),
                            512 * nn:512 * (nn + 1)], y_sb[:])

        # software pipeline: hide stats latency behind next image's proj
        emit_S1(0)
        emit_S1(1)
        emit_A(0)
        emit_S1(2)
        emit_P(0)
        emit_A(1)
        emit_S1(3)
        emit_P(1)
        emit_A(2)
        emit_P(2)
        emit_A(3)
        emit_P(3)
    nc.compile()
    return nc


def _host_tables():
    inv_freq = 1.0 / (10000.0 ** (np.arange(0, 16, dtype=np.float64) * 2 / 32))
    wpos = np.arange(W, dtype=np.float64)
    ang = wpos[:, None] * inv_freq[None, :]          # [28, 16]
    cosw = np.cos(ang).astype(np.float32)
    sinw = np.sin(ang).astype(np.float32)
    # per-token (within image) tables, replicated per head:
    # C block = [cos, cos]; S block = [-sin, +sin]
    cblk = np.concatenate([cosw, cosw], axis=1)       # [28, 32]
    sblk = np.concatenate([-sinw, sinw], axis=1)      # [28, 32]
    crow = np.tile(cblk, (1, HEADS))                  # [28, 512]
    srow = np.tile(sblk, (1, HEADS))
    ctab = np.tile(crow, (H, 1)).reshape(TPI, 512)    # rows t=r*28+w -> w pattern
    stab = np.tile(srow, (H, 1)).reshape(TPI, 512)
    at = np.zeros((TT, 28), dtype=np.float32)
    p = np.arange(TT)
    at[p, p % W] = ALPHA    # alpha folded into the img row-sum one-hot
    idn = np.eye(128, dtype=np.float32)
    return (ctab.astype(NPBF), stab.astype(NPBF), at.astype(NPBF),
            idn.astype(NPBF))


def _make_in_maps(x, input_img, qkv_w, o_w):
    x = np.ascontiguousarray(np.asarray(x, dtype=np.float32)).astype(NPBF)
    input_img = np.ascontiguousarray(
        np.asarray(input_img, dtype=np.float32)).astype(NPBF)
    qkv_w = np.asarray(qkv_w, dtype=np.float32)
    o_w = np.ascontiguousarray(np.asarray(o_w, dtype=np.float32)).astype(NPBF)
    ctab, stab, at, idn = _host_tables()
    wqk = np.ascontiguousarray(
        np.concatenate([qkv_w[:, 0:HID], qkv_w[:, 2 * HID:3 * HID]],
                       axis=1)).astype(NPBF)
    wv = np.ascontiguousarray(qkv_w[:, HID:2 * HID]).astype(NPBF)

    in_maps = []
    for c in range(N_CORES):
        in_maps.append({
            "x": x[B_CORE * c:B_CORE * (c + 1)].reshape(TOK, HID),
            "img": input_img[B_CORE * c:B_CORE * (c + 1)].reshape(TOK, HID),
            "wqk": wqk, "wv": wv, "wo": o_w,
            "at": at, "idn": idn, "ctab": ctab, "stab": stab,
        })
    return in_maps


def kernel(x, input_img, qkv_w, o_w, gn_w, gn_b):
    gn_w = np.asarray(gn_w, dtype=np.float32)
    gn_b = np.asarray(gn_b, dtype=np.float32)

    key = (tuple(gn_w.tolist()), tuple(gn_b.tolist()))
    if key not in _CACHE:
        _CACHE[key] = _build_program(gn_w, gn_b)
    nc = _CACHE[key]

    in_maps = _make_in_maps(x, input_img, qkv_w, o_w)
    res = run_bass_kernel_spmd(nc, in_maps, list(range(N_CORES)))
    out = np.concatenate(
        [np.asarray(res.results[c]["y"]).astype(np.float32).reshape(
            B_CORE, H, W, HID) for c in range(N_CORES)], axis=0)
    return out
